# revision 57
# baseline (speedup 1.0000x reference)
"""Trainium2 Bass kernel for nn_CorrTorch: 27-shift 3D correlation + 1x1x1 conv.

Math (B=1, C=32, D=H=W=64, NOFF=27):
  cv[(k,c), s] = x1[c,s] * pad(x2)[c, s + off_k] / sqrt(C)    (864 x 64^3)
  out[o, s]    = sum_{k,c} conv_w[o, k*32+c] * cv[(k,c), s] + conv_b[o]

Sharding: D axis split across 8 cores (8 planes each), 1-voxel halo baked into
the per-core x2 slab on the host. No collectives.

Per-core device strategy (v6, "quartered pairs"):
  - Partition layout packs 32 channels x 4 spatial QUARTERS (16-row bands of
    each plane) into all 128 partitions: partition 32q+c holds channel c,
    rows [16q,16q+16) of the plane.  One product instruction per shift cell
    (dz,dy,dx) covers a PAIR of planes in 2048 columns -- 27*1024 = 27648
    instruction-columns per plane, the per-column floor (engine cost is
    per-column, independent of partition count), with per-instruction
    overhead amortized over two planes.
  - No baked-shift replication: x1 is stored once (quarter-split pair tiles
    [128, 2, 1024]) and x2 lives in ONE persistent [128, 10, 18, 66] SBUF
    slab (quarter bands with a 2-row halo), DMA'd plane-by-plane; total DMA
    is ~8.6MB per core (vs 14.7 for the baked 9-pass scheme).  dy/dx shifts
    are AP row/col offsets; dz/plane are AP plane offsets into the slab.
  - Products split DVE (2x bf16, 0.52 ns/col) / GPSIMD (1.98 ns/col):
    GPSIMD takes cells {1,4,7,13}, rows [0,10) of cell 22, and cell 19 in
    row-halves; DVE the rest (~43.8k cols/pair vs GP ~11.5k); both end
    together (~24.2us/pair).
  - 1x1 conv runs transposed on the PE per 128-site chunk (2 rows x 64):
    128-partition matmuls with quarter-masked weights accumulate 27 cells
    into psum[site, o].  ~19.4us/pair, off the critical path.
  - PSUM: chunk (h, q, ul) -> bank ul//2, col (h*8 + 4*(ul%2) + q)*27, so a
    pair's 8 plane-banks pack into 4 psum banks (432 f32 each) and pairs
    double-buffer.  Cell 26's row-halves close banks {0,1} before {2,3} so
    eviction overlaps the tail.
  - ScalarE evicts PSUM -> a [128, 1728] f32 stage tile per pair (DVE does
    the last pair, it is ~2x faster per copy and idle by then); one DMA per
    2 banks writes HBM in a scrambled [pair, p, bank, slot, o] layout the
    host untangles (bias is added on the host).
"""

import numpy as np
import ml_dtypes

import concourse.bass as bass
import concourse.mybir as mybir
import concourse.tile as tile
from concourse.bass_utils import run_bass_kernel_spmd

C = 32
D = 64
H = 64
W = 64
NOFF = 27
NCORES = 8
DLOC = D // NCORES          # 8 output planes per core
NPAIR = DLOC // 2           # 4 plane-pairs per core
NSLAB = DLOC + 2            # 10 padded x2 planes per core
WP = W + 2                  # 66
NQ = 4                      # spatial quarters (16-row bands)
QR = H // NQ                # 16 rows per quarter
QROWS = QR + 2              # 18 rows per x2 quarter band (1-row halo each side)
QPF = QROWS * WP            # 1188 x2 elements per quarter-band per partition
QTN = QR * W                # 1024 sites per quarter per plane
NCELL = 27
CHUNK = 128                 # spatial columns per transposed matmul (2 rows)
NBANK = 4                   # psum bank tiles per pair
BCOLS = 16 * NOFF           # 432 psum columns per bank tile (16 slots)

GP_FULL = (1, 4, 7, 13)
GP_PART = 22                # rows [0,10) on GPSIMD, [10,16) on DVE
GP_PART_ROWS = 10
GP_LAST = 19                # GPSIMD, emitted as row-halves
DVE_LAST = 26               # DVE, row-halves carry the psum stop flags

BF16 = mybir.dt.bfloat16
F32 = mybir.dt.float32

_wsplit_ctr = [0]


def _split_sync_waits(nc, max_waits=1):
    """Walrus in this container accepts at most one sync wait per instruction.
    Hoist excess waits onto NoOp instructions inserted just before, on the
    same engine (same-engine program order preserves the semantics)."""
    for fn in nc.m.functions:
        for bb in fn.blocks:
            new = []
            changed = False
            for ins in bb.instructions:
                si = ins.sync_info
                if si is not None and len(si.on_wait) > max_waits:
                    waits = list(si.on_wait)
                    excess, keep = waits[:-max_waits], waits[-max_waits:]
                    for i in range(0, len(excess), max_waits):
                        _wsplit_ctr[0] += 1
                        new.append(
                            mybir.InstNoOp(
                                name=f"wsplit-{_wsplit_ctr[0]}",
                                engine=ins.engine,
                                sync_info=mybir.SyncInfo(
                                    on_wait=excess[i : i + max_waits], on_update=[]
                                ),
                            )
                        )
                    ins.sync_info = mybir.SyncInfo(
                        on_wait=keep, on_update=list(si.on_update)
                    )
                    changed = True
                new.append(ins)
            if changed:
                bb.instructions = new


def _plane_schedule():
    """Per-pair product worklists [(cell, engine, h0, h1, y0, y1)] in
    emission order (per-engine program order is what matters) and the PE
    consumption order interleaved by estimated completion time (DVE full
    pair-cell ~1.13us, GPSIMD ~4.16us)."""
    dve_order = [0, 2, 3, 5, 6, 8, 9, (GP_PART, 0, 2, GP_PART_ROWS, QR),
                 10, 11, 12, 14, 15, 16, 17, 18, 20, 21,
                 (DVE_LAST, 0, 2, 0, QR // 2), 23, 24, 25,
                 (DVE_LAST, 0, 2, QR // 2, QR)]
    gp_order = [1, 4, (GP_PART, 0, 2, 0, GP_PART_ROWS), 7, 13,
                (GP_LAST, 0, 2, 0, QR // 2), (GP_LAST, 0, 2, QR // 2, QR)]

    def expand(order, eng_name):
        out = []
        for item in order:
            l, h0, h1, y0, y1 = (
                item if isinstance(item, tuple) else (item, 0, 2, 0, QR)
            )
            out.append((l, eng_name, h0, h1, y0, y1))
        return out

    prods = expand(dve_order, "dve") + expand(gp_order, "pool")
    # Pair 0: split the first cell on each engine by plane/row-half so
    # products start on the first chunks of the x1/x2 DMAs.
    first0 = [(0, 0, 1, 0, 8), (0, 0, 1, 8, QR), (2, 0, 1, 0, QR),
              (0, 1, 2, 0, QR), (2, 1, 2, 0, QR)]
    first1 = [(1, 0, 1, 0, 8), (1, 0, 1, 8, QR), (4, 0, 1, 0, QR),
              (1, 1, 2, 0, QR), (4, 1, 2, 0, QR)]
    prods0 = expand(first0 + dve_order[2:], "dve") + expand(
        first1 + gp_order[2:], "pool"
    )
    pe_order = [0, 2, 3, 1, 5, 6, 8, 9, 4, 10, 11, GP_PART, 12, 14, 15, 7,
                16, 17, 18, 20, 13, 21, (DVE_LAST, 0), 23, 24, GP_LAST, 25,
                (DVE_LAST, 1)]
    cells = [e[0] if isinstance(e, tuple) else e for e in pe_order]
    assert sorted(set(cells)) == list(range(NCELL))
    return prods0, prods, pe_order


def build_program():
    nc = bass.Bass()

    x1r = nc.dram_tensor("x1r", [DLOC, 128, QTN], BF16, kind="ExternalInput")
    x2r = nc.dram_tensor("x2r", [NSLAB, 128, QPF], BF16, kind="ExternalInput")
    # Weights per (cell, quarter): only quarter q's 32 partitions are nonzero,
    # so a full 128-partition matmul picks out one quarter's contraction (the
    # PE rejects base partition 96, so 32-deep per-quarter matmuls are out).
    wts = nc.dram_tensor(
        "wts", [128, NCELL * NQ * NOFF], BF16, kind="ExternalInput"
    )
    # Output staged/stored as bf16: quantization error (<=0.12% of absmax)
    # is far inside the tolerance, and it halves eviction DMA time.
    out = nc.dram_tensor(
        "out", [NPAIR, 128, NBANK * BCOLS], BF16, kind="ExternalOutput"
    )

    prods0, prods, pe_order = _plane_schedule()

    with tile.TileContext(nc) as tc:
        with (
            tc.tile_pool(name="wt", bufs=1) as wt_pool,
            tc.tile_pool(name="x2", bufs=1) as x2_pool,
            tc.tile_pool(name="x1", bufs=3) as x1_pool,
            tc.tile_pool(name="cv", bufs=8) as cv_pool,
            tc.tile_pool(name="stage", bufs=2) as stage_pool,
            tc.tile_pool(name="psum", bufs=2, space="PSUM") as psum_pool,
        ):
            # Persistent x2 slab: all 10 quarter-band planes stay resident
            # (23.8KB/partition); products read [plane, row, col] windows.
            slab = x2_pool.tile([128, NSLAB, QROWS, WP], BF16, name="x2slab")
            x1t = {}

            def load_x1_pair(p):
                t = x1_pool.tile([128, 2, QTN], BF16, tag="x1q", name="x1q")
                nc.scalar.dma_start(out=t[:, 0], in_=x1r[2 * p])
                nc.scalar.dma_start(out=t[:, 1], in_=x1r[2 * p + 1])
                x1t[p] = t

            def load_x2_plane(pl):
                nc.sync.dma_start(out=slab[:, pl], in_=x2r[pl])

            # Startup: plane-0/pair-0 first chunks land first so the split
            # first cells can start early.
            # The (mostly-zero) 746KB weight tile is only needed by the first
            # matmul (~9us in, PE has slack in pair 0), so it loads LAST --
            # queueing it earlier stalls the x1/x2 loads products wait on.
            t0 = x1_pool.tile([128, 2, QTN], BF16, tag="x1q", name="x1q")
            nc.sync.dma_start(out=slab[:, 0, 0:11, :], in_=x2r[0][:, 0 : 11 * WP])
            nc.scalar.dma_start(out=t0[:, 0, 0:512], in_=x1r[0][:, 0:512])
            nc.sync.dma_start(out=slab[:, 0, 11:QROWS, :], in_=x2r[0][:, 11 * WP : QPF])
            nc.scalar.dma_start(out=t0[:, 0, 512:QTN], in_=x1r[0][:, 512:QTN])
            load_x2_plane(1)
            nc.scalar.dma_start(out=t0[:, 1], in_=x1r[1])
            x1t[0] = t0
            load_x2_plane(2)
            load_x2_plane(3)
            wt_tile = wt_pool.tile([128, NCELL * NQ * NOFF], BF16)
            nc.sync.dma_start(out=wt_tile[:], in_=wts[:])
            load_x1_pair(1)

            for p in range(NPAIR):
                d0 = 2 * p
                for pl in (d0 + 4, d0 + 5):
                    if pl < NSLAB:
                        load_x2_plane(pl)
                if p + 2 < NPAIR:
                    load_x1_pair(p + 2)

                pts = [
                    psum_pool.tile([128, BCOLS], F32, tag=f"ps{b}", name=f"ps{b}")
                    for b in range(NBANK)
                ]

                seg_tiles = {}
                for l, eng_name, h0, h1, y0, y1 in (prods0 if p == 0 else prods):
                    dz, dy, dx = l // 9, (l // 3) % 3, l % 3
                    ncols = (h1 - h0) * (y1 - y0) * W
                    cvt = cv_pool.tile(
                        [128, ncols],
                        BF16,
                        tag=f"cv_{eng_name}{ncols}",
                        name="cvseg",
                        bufs={2 * QTN: 14 if eng_name == "dve" else 3,
                              QTN: 3}.get(ncols, 3),
                    )
                    x1s = x1t[p][:, h0:h1, y0 * W : y1 * W]
                    x2s = slab[
                        :, d0 + h0 + dz : d0 + h1 + dz, dy + y0 : dy + y1, dx : dx + W
                    ]
                    eng = nc.gpsimd if eng_name == "pool" else nc.vector
                    eng.tensor_mul(out=cvt[:], in0=x1s, in1=x2s)
                    seg_tiles.setdefault(l, []).append((h0, h1, y0, y1, cvt))

                # Transposed 1x1 conv: chunk (h, q, ul) -> psum bank ul//2,
                # col (h*8 + 4*(ul%2) + q)*27.  One accumulation group per
                # bank: first matmul emitted per bank carries start (zeroes
                # the whole 2KB bank), last emitted carries stop.
                n_emitted = {b: 0 for b in range(NBANK)}
                PER_BANK = NCELL * 16
                for item in pe_order:
                    l, segsel = item if isinstance(item, tuple) else (item, None)
                    segs = (
                        seg_tiles[l] if segsel is None else [seg_tiles[l][segsel]]
                    )
                    for h0, h1, y0, y1, cvt in segs:
                        for h in range(h0, h1):
                            for ul in range(y0 // 2, y1 // 2):
                                for q in range(NQ):
                                    b = ul // 2
                                    slot = h * 8 + 4 * (ul % 2) + q
                                    g0 = ((h - h0) * (y1 - y0) + 2 * ul - y0) * W
                                    n_emitted[b] += 1
                                    w0 = (l * NQ + q) * NOFF
                                    nc.tensor.matmul(
                                        pts[b][:, NOFF * slot : NOFF * (slot + 1)],
                                        lhsT=cvt[:, g0 : g0 + CHUNK],
                                        rhs=wt_tile[:, w0 : w0 + NOFF],
                                        start=(n_emitted[b] == 1),
                                        stop=(n_emitted[b] == PER_BANK),
                                    )

                # Eviction: PSUM -> SBUF stage -> HBM.  Mid-kernel the copies
                # ride the idle ACT engine (they overlap the next pair's
                # products); the last pair's go on the then-idle DVE.
                stage = stage_pool.tile(
                    [128, NBANK * BCOLS], BF16, tag="stage", name="stage"
                )
                last = p == NPAIR - 1
                for half in range(2):
                    b0, b1 = 2 * half, 2 * half + 1
                    nc.scalar.copy(stage[:, BCOLS * b0 : BCOLS * (b0 + 1)], pts[b0][:])
                    # Last pair: the halves' second copies ride the idle DVE
                    # concurrently with ACT so each half closes ~0.55us sooner.
                    ev1 = nc.vector.tensor_copy if last else nc.scalar.copy
                    ev1(stage[:, BCOLS * b1 : BCOLS * (b1 + 1)], pts[b1][:])
                    nc.sync.dma_start(
                        out=out[p][:, 2 * half * BCOLS : (2 * half + 2) * BCOLS],
                        in_=stage[:, 2 * half * BCOLS : (2 * half + 2) * BCOLS],
                    )

    _split_sync_waits(nc)
    return nc


_PROGRAM = None


def _get_program():
    global _PROGRAM
    if _PROGRAM is None:
        _PROGRAM = build_program()
    return _PROGRAM


def _prep_inputs(in1, in2, conv_w):
    """Build the 8 per-core input maps (bf16 quarter-split layout on host)."""
    x1 = np.ascontiguousarray(np.asarray(in1, np.float32).reshape(C, D, H, W))
    x2 = np.ascontiguousarray(np.asarray(in2, np.float32).reshape(C, D, H, W))
    scale = 1.0 / np.sqrt(np.float32(C))
    Wk = (np.asarray(conv_w, np.float32) * scale).reshape(NOFF, NOFF, C)  # [o,l,c]

    # wts[32q+c, (l*4+q')*27+o] = Wk[o, l, c] if q'==q else 0
    wts = np.zeros((128, NCELL, NQ, NOFF), np.float32)
    wlc = Wk.transpose(2, 1, 0)  # [c, l, o]
    for q in range(NQ):
        wts[32 * q : 32 * q + C, :, q, :] = wlc
    wts = wts.reshape(128, NCELL * NQ * NOFF).astype(ml_dtypes.bfloat16)

    # Global zero-padded x2: pad plane/row/col index = global index + 1.
    x2p = np.zeros((C, D + 2, H + 2, WP), np.float32)
    x2p[:, 1 : D + 1, 1 : H + 1, 1 : W + 1] = x2

    in_maps = []
    for m in range(NCORES):
        # x2 quarter bands: [NSLAB, 32q+c, 18*66]
        slab = x2p[:, DLOC * m : DLOC * m + NSLAB]  # [C, 10, 66, 66]
        bands = np.stack(
            [slab[:, :, QR * q : QR * q + QROWS, :] for q in range(NQ)], axis=0
        )  # [4, C, 10, 18, 66]
        x2q = (
            bands.reshape(NQ, C, NSLAB, QPF)
            .transpose(2, 0, 1, 3)
            .reshape(NSLAB, 128, QPF)
            .astype(ml_dtypes.bfloat16)
        )

        # x1 quarters: [DLOC, 32q+c, 1024]
        x1c = x1[:, DLOC * m : DLOC * (m + 1)].reshape(C, DLOC, NQ, QTN)
        x1q = (
            x1c.transpose(1, 2, 0, 3)
            .reshape(DLOC, 128, QTN)
            .astype(ml_dtypes.bfloat16)
        )

        in_maps.append(
            {
                "x1r": np.ascontiguousarray(x1q),
                "x2r": np.ascontiguousarray(x2q),
                "wts": np.ascontiguousarray(wts),
            }
        )
    return in_maps


def _site_perm():
    """src[h, site] = flat (bank*16 + slot)*128 + p index holding that
    (plane-in-pair, site)."""
    s = np.arange(H * W)
    q, j = s // QTN, s % QTN
    ul, pp = j // CHUNK, j % CHUNK
    bank, upar = ul // 2, ul % 2
    src = np.empty((2, H * W), np.int64)
    for h in range(2):
        slot = h * 8 + 4 * upar + q
        src[h] = (bank * 16 + slot) * CHUNK + pp
    return src


def kernel(in1, in2, conv_w, conv_b):
    nc = _get_program()
    in_maps = _prep_inputs(in1, in2, conv_w)
    res = run_bass_kernel_spmd(nc, in_maps, core_ids=list(range(NCORES)))
    src = _site_perm()
    outs = []
    for r in res.results:
        raw = r["out"].reshape(NPAIR, 128, NBANK * 16, NOFF)
        # [pair, p, (bank, slot), o] -> [o, pair, (bank*16+slot)*128+p]
        flat = raw.transpose(3, 0, 2, 1).reshape(NOFF, NPAIR, NBANK * 16 * 128)
        per = flat[:, :, src]  # [o, pair, h, site]
        per = per.reshape(NOFF, DLOC, H, W)
        outs.append(per)
    full = np.concatenate(outs, axis=1)  # [27, 64, 64, 64]
    full = full + np.asarray(conv_b, np.float32)[:, None, None, None]
    return full[None].astype(np.float32)  # [1, 27, 64, 64, 64]
